# revision 14
# baseline (speedup 1.0000x reference)
"""Trainium2 Bass kernel for a 50-step autoregressive MLP rollout.

reference semantics (per batch row b):
    state = x[b, 0, 2:9]                       # 7 state vars
    for t in range(50):
        u = x[b, t, 0:2]                       # 2 controls
        h1 = tanh([u, state] @ W1 + b1)        # [9] -> [256]
        h2 = tanh(h1 @ W2 + b2)                # [256] -> [256]
        d  = h2 @ W3 + b3                      # [256] -> [7]
        state = state + 0.02 * d
        out[b, t] = state

Strategy: pure data parallel over batch across 8 NeuronCores (4096 rows each).
On-chip layout is feature-major ([feature, batch]) so the MLP weights are the
stationary matmul operand and the recurrence never transposes activations.
Matmuls run as float32r (full-rate fp32 path).  All SBUF access patterns keep
base partitions 32-aligned (HW quadrant rule): each step's control pair and
state vector live in their own quadrant slot (4 steps per [128, B] tile), and
the first-layer matmul accumulates a control-part (K=2) and a state-part (K=7)
into the same PSUM bank.  b1/b2/b3 are zeros for this problem (spec
fill=zeros) and asserted so.
"""

import numpy as np

# problem constants (hardcoded per harness contract)
B_TOTAL = 32768
N_CORES = 8
B_CORE = B_TOTAL // N_CORES          # 4096
H = 50                                # horizon
F = 9                                 # features per step (2 controls + 7 states)
NCTRL = 2
NST = 7
HID = 256
DT = 0.02
NTILE = 512                           # batch columns per PSUM tile
SH_STEPS = 18                         # steps per output-transpose chunk (18*7=126 rows)

_CACHE = {}


def _build(b_core=B_CORE, horizon=H):
    import concourse.bacc as bacc
    import concourse.mybir as mybir
    import concourse.tile as tile

    f32 = mybir.dt.float32
    f32r = mybir.dt.float32r
    Tanh = mybir.ActivationFunctionType.Tanh

    nb = b_core // NTILE              # N-tiles per step
    n_groups = (horizon + 4) // 4     # 4 steps per quadrant-slot tile
    n_sh = (horizon + SH_STEPS - 1) // SH_STEPS
    n_blk = b_core // 128             # batch blocks for transposes
    xcols = horizon * F

    nc = bacc.Bacc("TRN2", target_bir_lowering=False, debug=False,
                   num_devices=N_CORES)

    x_d = nc.dram_tensor("x", [b_core, xcols], f32, kind="ExternalInput").ap()
    w1_d = nc.dram_tensor("w1", [F, HID], f32r, kind="ExternalInput").ap()
    w2_d = nc.dram_tensor("w2", [HID, HID], f32r, kind="ExternalInput").ap()
    w3_d = nc.dram_tensor("w3dt", [HID, NST], f32r, kind="ExternalInput").ap()
    id_d = nc.dram_tensor("ident", [128, 128], f32, kind="ExternalInput").ap()
    out_d = nc.dram_tensor("out", [b_core, horizon * NST], f32,
                           kind="ExternalOutput").ap()
    # internal DRAM staging: transposed controls and state history
    ust_d = nc.dram_tensor("ustage", [128, b_core], f32r,
                           kind="Internal").ap()
    hst_d = nc.dram_tensor("hstage", [NST * horizon, b_core], f32,
                           kind="Internal").ap()

    with tile.TileContext(nc) as tc:
        with (
            tc.tile_pool(name="persist", bufs=1) as pp,
            tc.tile_pool(name="xst", bufs=2) as xp,
            tc.tile_pool(name="uT", bufs=2) as up_,
            tc.tile_pool(name="sT", bufs=2) as sp_,
            tc.tile_pool(name="h1p", bufs=2) as h1p,
            tc.tile_pool(name="h2p", bufs=2) as h2p,
            tc.tile_pool(name="shb", bufs=2) as shp,
            tc.tile_pool(name="ostg", bufs=4) as op_,
            tc.tile_pool(name="psA", bufs=2, space="PSUM") as psA,
            tc.tile_pool(name="psB", bufs=1, space="PSUM") as psB,
            tc.tile_pool(name="psS", bufs=2, space="PSUM") as psS,
        ):
            # ---- persistent tiles ----
            w1c = pp.tile([128, HID], f32r, tag="w1c")   # W1[0:2] at 4 quadrants
            w1s = pp.tile([128, HID], f32r, tag="w1s")   # W1[2:9] at 4 quadrants
            w2k0 = pp.tile([128, HID], f32r, tag="w2k0")
            w2k1 = pp.tile([128, HID], f32r, tag="w2k1")
            w3sb = pp.tile([128, 2 * NST], f32r, tag="w3sb")
            ident = pp.tile([128, 128], f32, tag="ident")
            ut = pp.tile([128, b_core], f32r, tag="ut")  # controls.T row 2t+f

            for k in range(4):
                nc.sync.dma_start(w1c[32 * k:32 * k + NCTRL, :], w1_d[0:NCTRL, :])
                nc.sync.dma_start(w1s[32 * k:32 * k + NST, :], w1_d[NCTRL:F, :])
            nc.sync.dma_start(w2k0[:, :], w2_d[0:128, :])
            nc.sync.dma_start(w2k1[:, :], w2_d[128:256, :])
            nc.sync.dma_start(w3sb[:, 0:NST], w3_d[0:128, :])
            nc.sync.dma_start(w3sb[:, NST:2 * NST], w3_d[128:256, :])
            nc.sync.dma_start(ident[:, :], id_d[:, :])

            ugroups = [None] * (n_groups + 1)
            sgroups = [None] * (n_groups + 1)

            def fill_u(g):
                tu = ugroups[g]
                for s in range(4):
                    t = 4 * g + s
                    if t < horizon:
                        # ustage row layout: fi*64 + t
                        usrc = ust_d.rearrange("(f t) b -> t f b", t=64)[t]
                        nc.sync.dma_start(tu[32 * s:32 * s + NCTRL, :], usrc)

            def alloc_group(g, fill=True):
                tu = up_.tile([128, b_core], f32r, name=f"uT_g{g}", tag="uT")
                ts_ = sp_.tile([128, b_core], f32r, name=f"sT_g{g}", tag="sT")
                ugroups[g], sgroups[g] = tu, ts_
                if fill:
                    fill_u(g)
                return tu, ts_

            _, st0 = alloc_group(0, fill=False)

            # ---- prologue: transpose controls (all t) and state0 ----
            blk_per_dma = min(4, n_blk)
            for q in range(n_blk // blk_per_dma):
                rows = blk_per_dma * 128
                xs = xp.tile([128, blk_per_dma * xcols], f32, tag="xs")
                src = x_d[q * rows:(q + 1) * rows, :].rearrange(
                    "(j p) c -> p j c", p=128)
                nc.sync.dma_start(
                    xs[:, :].rearrange("p (j c) -> p j c", c=xcols), src)
                cw = blk_per_dma * 128
                # ut row layout: fi*64 + t (keeps all SBUF bases 32-aligned)
                for fi in range(NCTRL):
                    pu = psS.tile([128, NTILE], f32, tag="small", name="pu")
                    for j in range(blk_per_dma):
                        xv = xs[:, j * xcols:(j + 1) * xcols].rearrange(
                            "p (t f) -> p t f", f=F)
                        nc.tensor.transpose(
                            pu[0:horizon, j * 128:(j + 1) * 128],
                            xv[:, :, fi], ident[:, :])
                    nc.vector.tensor_copy(
                        ut[64 * fi:64 * fi + horizon, q * cw:(q + 1) * cw],
                        pu[0:horizon, 0:cw])
                ps0 = psS.tile([128, NTILE], f32, tag="small", name="ps0")
                for j in range(blk_per_dma):
                    nc.tensor.transpose(
                        ps0[0:NST, j * 128:(j + 1) * 128],
                        xs[:, j * xcols + NCTRL:j * xcols + F], ident[:, :])
                nc.vector.tensor_copy(st0[0:NST, q * cw:(q + 1) * cw],
                                      ps0[0:NST, 0:cw])
            for fi in range(NCTRL):
                nc.sync.dma_start(ust_d[64 * fi:64 * fi + horizon, :],
                                  ut[64 * fi:64 * fi + horizon, :])
            fill_u(0)

            # ---- epilogue helper: batch-major output for steps chunk k ----
            def emit_output(k):
                r0 = k * SH_STEPS
                nrows = (min(horizon, r0 + SH_STEPS) - r0) * NST
                shb = shp.tile([128, b_core], f32, tag="shb", name=f"shb{k}")
                nc.sync.dma_start(shb[0:nrows, :],
                                  hst_d[r0 * NST:r0 * NST + nrows, :])
                for blk in range(n_blk):
                    pt = psS.tile([128, NTILE], f32, tag="small", name=f"pt{k}")
                    nc.tensor.transpose(
                        pt[0:128, 0:nrows],
                        shb[0:nrows, blk * 128:(blk + 1) * 128],
                        ident[0:nrows, 0:nrows])
                    ost = op_.tile([128, 128], f32, tag="ost")
                    nc.vector.tensor_copy(ost[:, 0:nrows], pt[0:128, 0:nrows])
                    nc.sync.dma_start(
                        out_d[blk * 128:(blk + 1) * 128,
                              r0 * NST:r0 * NST + nrows],
                        ost[:, 0:nrows])

            # ---- main scan ----
            for t in range(horizon):
                g, s = divmod(t, 4)
                g2, s2 = divmod(t + 1, 4)
                tu, ts_ = ugroups[g], sgroups[g]
                for j in range(nb):
                    c0, c1 = j * NTILE, (j + 1) * NTILE
                    ph1 = psA.tile([128, 2 * NTILE], f32, tag="ph1", name="ph1")
                    for m in range(2):
                        nc.tensor.matmul(
                            ph1[:, m * NTILE:(m + 1) * NTILE],
                            w1c[32 * s:32 * s + NCTRL,
                                m * 128:(m + 1) * 128],
                            tu[32 * s:32 * s + NCTRL, c0:c1],
                            start=True, stop=False, tile_position=(32 * s, 0))
                        nc.tensor.matmul(
                            ph1[:, m * NTILE:(m + 1) * NTILE],
                            w1s[32 * s:32 * s + NST,
                                m * 128:(m + 1) * 128],
                            ts_[32 * s:32 * s + NST, c0:c1],
                            start=False, stop=True, tile_position=(32 * s, 0))
                    h1t = h1p.tile([128, 2 * NTILE], f32r, tag="h1")
                    nc.scalar.activation(h1t[:, :], ph1[:, :], Tanh)

                    ph2 = psB.tile([128, 2 * NTILE], f32, tag="ph2", name="ph2")
                    for m in range(2):
                        nc.tensor.matmul(
                            ph2[:, m * NTILE:(m + 1) * NTILE],
                            w2k0[:, m * 128:(m + 1) * 128],
                            h1t[:, 0:NTILE],
                            start=True, stop=False)
                        nc.tensor.matmul(
                            ph2[:, m * NTILE:(m + 1) * NTILE],
                            w2k1[:, m * 128:(m + 1) * 128],
                            h1t[:, NTILE:2 * NTILE],
                            start=False, stop=True)
                    h2t = h2p.tile([128, 2 * NTILE], f32r, tag="h2")
                    nc.scalar.activation(h2t[:, :], ph2[:, :], Tanh)

                    pd = psS.tile([128, NTILE], f32, tag="small", name="pd")
                    nc.tensor.matmul(pd[0:NST, :], w3sb[:, 0:NST],
                                     h2t[:, 0:NTILE],
                                     start=True, stop=False)
                    nc.tensor.matmul(pd[0:NST, :], w3sb[:, NST:2 * NST],
                                     h2t[:, NTILE:2 * NTILE],
                                     start=False, stop=True)

                    if j == 0 and g2 > g:
                        alloc_group(g2)
                    # state(t+1) = state(t) + d  (W3 pre-scaled by DT on host)
                    nc.vector.tensor_add(
                        sgroups[g2][32 * s2:32 * s2 + NST, c0:c1],
                        pd[0:NST, :],
                        ts_[32 * s:32 * s + NST, c0:c1])

                # record state(t+1) as output row t of the history (DRAM stage)
                nc.sync.dma_start(
                    hst_d[NST * t:NST * (t + 1), :],
                    sgroups[g2][32 * s2:32 * s2 + NST, :].bitcast(f32))

                if t > 0 and t % SH_STEPS == 0:
                    emit_output(t // SH_STEPS - 1)

            emit_output(n_sh - 1)

    nc.compile()
    return nc


def _get_nc(b_core=B_CORE, horizon=H):
    key = (b_core, horizon)
    if key not in _CACHE:
        _CACHE[key] = _build(b_core, horizon)
    return _CACHE[key]


def _run(x, W1, b1, W2, b2, W3, b3, **spmd_kwargs):
    import concourse.bass_utils as bass_utils

    x = np.ascontiguousarray(np.asarray(x, dtype=np.float32))
    W1 = np.ascontiguousarray(np.asarray(W1, dtype=np.float32))
    W2 = np.ascontiguousarray(np.asarray(W2, dtype=np.float32))
    W3 = np.ascontiguousarray(np.asarray(W3, dtype=np.float32))
    for b in (b1, b2, b3):
        assert not np.any(np.asarray(b)), "kernel built for zero biases"

    nc = _get_nc()
    w3dt = np.ascontiguousarray(DT * W3)
    ident = np.eye(128, dtype=np.float32)
    xr = x.reshape(B_TOTAL, H * F)

    in_maps = []
    for c in range(N_CORES):
        in_maps.append({
            "x": xr[c * B_CORE:(c + 1) * B_CORE],
            "w1": W1, "w2": W2, "w3dt": w3dt, "ident": ident,
        })
    res = bass_utils.run_bass_kernel_spmd(nc, in_maps,
                                          core_ids=list(range(N_CORES)),
                                          **spmd_kwargs)
    out = np.concatenate(
        [res.results[c]["out"].reshape(B_CORE, H, NST) for c in range(N_CORES)],
        axis=0)
    return out, res


def kernel(x, W1, b1, W2, b2, W3, b3):
    out, _ = _run(x, W1, b1, W2, b2, W3, b3)
    return out


# revision 39
# speedup vs baseline: 103.9249x; 103.9249x over previous
"""Trainium2 Bass kernel for a 50-step autoregressive MLP rollout.

reference semantics (per batch row b):
    state = x[b, 0, 2:9]                       # 7 state vars
    for t in range(50):
        u = x[b, t, 0:2]                       # 2 controls
        h1 = tanh([u, state] @ W1 + b1)        # [9] -> [256]
        h2 = tanh(h1 @ W2 + b2)                # [256] -> [256]
        d  = h2 @ W3 + b3                      # [256] -> [7]
        state = state + 0.02 * d
        out[b, t] = state

Strategy: pure data parallel over batch across 8 NeuronCores (4096 rows each).
On-chip layout is feature-major ([feature, batch]) so the MLP weights are the
stationary matmul operand and the recurrence never transposes activations.
Matmuls run as float32r (full-rate fp32 path).  Each step's 9-feature input
vector lives in one quadrant slot of a [128, B] tile: state in rows 32s..32s+7
(written by the DVE update at an aligned base) and the two controls in rows
32s+7..32s+9 (written by DMA, which may target unaligned partitions; compute
engines may not).  The first MLP layer is then a single K=9 matmul per output
chunk.  Controls are transposed on-chip with the PE at startup and staged
through DRAM so each step's pair can be fetched to its slot with a plain DMA.
The state history is staged to DRAM per step and transposed back to
batch-major in chunks that overlap the scan.  b1/b2/b3 are zeros for this
problem (spec fill=zeros) and asserted so.
"""

import numpy as np

B_TOTAL = 32768
N_CORES = 8
B_CORE = B_TOTAL // N_CORES          # 4096
H = 50
F = 9
NCTRL = 2
NST = 7
HID = 256
DT = 0.02
NTILE = 512

_CACHE = {}


def _build(b_core=B_CORE, horizon=H, psa_bufs=1, psb_bufs=2, pd_tag="own", pt_tag="sm",
           chunks=(18, 18, 8, 6), spread=1, reps=1):
    import concourse.bacc as bacc
    import concourse.mybir as mybir
    import concourse.tile as tile

    f32 = mybir.dt.float32
    f32r = mybir.dt.float32r
    Tanh = mybir.ActivationFunctionType.Tanh

    nb = b_core // NTILE
    n_groups = (horizon + 4) // 4
    n_blk = b_core // 128
    xcols = horizon * F
    # output chunks (in steps); each must fit 128 sbuf rows (<=18)
    chunks = [c for c in chunks]
    while sum(chunks) > horizon:
        chunks[-1] -= 1
        if chunks[-1] == 0:
            chunks.pop()
    if sum(chunks) < horizon:
        chunks.append(horizon - sum(chunks))
    cstart = [sum(chunks[:i]) for i in range(len(chunks))]

    nc = bacc.Bacc("TRN2", target_bir_lowering=False, debug=False,
                   num_devices=N_CORES)

    x_d = nc.dram_tensor("x", [b_core, xcols], f32, kind="ExternalInput").ap()
    w1_d = nc.dram_tensor("w1", [F, HID], f32r, kind="ExternalInput").ap()
    w2_d = nc.dram_tensor("w2", [HID, HID], f32r, kind="ExternalInput").ap()
    w3_d = nc.dram_tensor("w3dt", [HID, NST], f32r, kind="ExternalInput").ap()
    id_d = nc.dram_tensor("ident", [128, 128], f32, kind="ExternalInput").ap()
    out_d = nc.dram_tensor("out", [b_core, horizon * NST], f32,
                           kind="ExternalOutput").ap()
    hst_d = nc.dram_tensor("hstage", [NST * horizon, b_core], f32,
                           kind="Internal").ap()
    ust_d = nc.dram_tensor("ustage", [128, b_core], f32r,
                           kind="Internal").ap()

    with tile.TileContext(nc) as tc:
        with (
            tc.tile_pool(name="persist", bufs=1) as pp,
            tc.tile_pool(name="xst", bufs=3) as xp,
            tc.tile_pool(name="sT", bufs=2) as sp_,
            tc.tile_pool(name="h1p", bufs=4) as h1p,
            tc.tile_pool(name="h2p", bufs=4) as h2p,
            tc.tile_pool(name="shb", bufs=2) as shp,
            tc.tile_pool(name="ostg", bufs=4) as op_,
            tc.tile_pool(name="psA", bufs=psa_bufs, space="PSUM") as psA,
            tc.tile_pool(name="psB", bufs=psb_bufs, space="PSUM") as psB,
            tc.tile_pool(name="psS", bufs=1, space="PSUM") as psS,
        ):
            w1sb = pp.tile([128, HID], f32r, tag="w1sb")
            w2k0 = pp.tile([128, HID], f32r, tag="w2k0")
            w2k1 = pp.tile([128, HID], f32r, tag="w2k1")
            w3sb = pp.tile([128, 2 * NST], f32r, tag="w3sb")
            ident = pp.tile([128, 128], f32, tag="ident")
            ut = pp.tile([128, b_core], f32r, tag="ut")  # controls.T row f*64+t

            for k in range(4):
                # per-quadrant W1, permuted to [state rows; control rows]
                nc.sync.dma_start(w1sb[32 * k:32 * k + NST, :], w1_d[NCTRL:F, :])
                nc.sync.dma_start(w1sb[32 * k + NST:32 * k + F, :],
                                  w1_d[0:NCTRL, :])
            nc.sync.dma_start(w2k0[:, :], w2_d[0:128, :])
            nc.sync.dma_start(w2k1[:, :], w2_d[128:256, :])
            nc.sync.dma_start(w3sb[:, 0:NST], w3_d[0:128, :])
            nc.sync.dma_start(w3sb[:, NST:2 * NST], w3_d[128:256, :])
            nc.sync.dma_start(ident[:, :], id_d[:, :])

            ust_v = ust_d.rearrange("(f t) b -> t f b", t=64)
            sgroups = [None] * (n_groups + 1)

            def fill_u(g):
                ts_ = sgroups[g]
                for s in range(4):
                    t = 4 * g + s
                    if t < horizon:
                        nc.sync.dma_start(
                            ts_[32 * s + NST:32 * s + F, :], ust_v[t])

            def alloc_group(g, fill=True):
                ts_ = sp_.tile([128, b_core], f32r, name=f"sT_g{g}", tag="sT")
                sgroups[g] = ts_
                if fill:
                    fill_u(g)
                return ts_

            for _rep in range(reps):
                st0 = alloc_group(0, fill=False)

                # ---- prologue: transpose controls (all t) and state0 ----
                bpd = min(4, n_blk)
                for q in range(n_blk // bpd):
                    rows = bpd * 128
                    cw = bpd * 128
                    xs = xp.tile([128, bpd * xcols], f32, tag="xs")
                    src = x_d[q * rows:(q + 1) * rows, :].rearrange(
                        "(j p) c -> p j c", p=128)
                    nc.sync.dma_start(
                        xs[:, :].rearrange("p (j c) -> p j c", c=xcols), src)
                    pu = psB.tile([128, 2 * NTILE], f32, tag="ph2", name="pu")
                    for fi in range(NCTRL):
                        for j in range(bpd):
                            xv = xs[:, j * xcols:(j + 1) * xcols].rearrange(
                                "p (t f) -> p t f", f=F)
                            nc.tensor.transpose(
                                pu[0:horizon,
                                   fi * NTILE + j * 128:fi * NTILE + (j + 1) * 128],
                                xv[:, :, fi], ident[:, :])
                    ps0 = psS.tile([128, NTILE], f32, tag="sm", name="ps0")
                    for j in range(bpd):
                        nc.tensor.transpose(
                            ps0[0:NST, j * 128:(j + 1) * 128],
                            xs[:, j * xcols + NCTRL:j * xcols + F],
                            ident[:, :])
                    for fi in range(NCTRL):
                        nc.vector.tensor_copy(
                            ut[64 * fi:64 * fi + horizon, q * cw:(q + 1) * cw],
                            pu[0:horizon, fi * NTILE:fi * NTILE + cw])
                    nc.vector.tensor_copy(st0[0:NST, q * cw:(q + 1) * cw],
                                          ps0[0:NST, 0:cw])
                for fi in range(NCTRL):
                    nc.sync.dma_start(ust_d[64 * fi:64 * fi + horizon, :],
                                      ut[64 * fi:64 * fi + horizon, :])
                fill_u(0)

                # ---- epilogue task queue: (chunk, blk) transposes ----
                pending = []
                shbs = {}

                BG = 4  # blocks per transpose group (BG*nrows <= 512: one PSUM bank)

                def start_chunk(k):
                    r0, nrows = cstart[k] * NST, chunks[k] * NST
                    shb = shp.tile([128, b_core], f32, tag="shb",
                                   name=f"shb{k}")
                    nc.sync.dma_start(shb[0:nrows, :],
                                      hst_d[r0:r0 + nrows, :])
                    shbs[k] = shb
                    pending.extend((k, gb) for gb in range(n_blk // BG))

                def emit_block(k, gb):
                    r0, nrows = cstart[k] * NST, chunks[k] * NST
                    shb = shbs[k]
                    ptp = {"sm": psS, "ph1": psA, "ph2": psB}[pt_tag]
                    pt = ptp.tile([128, 2 * NTILE], f32, tag=pt_tag, name="pt")
                    for i in range(BG):
                        blk = gb * BG + i
                        nc.tensor.transpose(
                            pt[0:128, i * nrows:(i + 1) * nrows],
                            shb[0:nrows, blk * 128:(blk + 1) * 128],
                            ident[0:nrows, 0:nrows])
                    ost = op_.tile([128, BG * 128], f32, tag="ost")
                    nc.vector.tensor_copy(ost[:, 0:BG * nrows],
                                          pt[0:128, 0:BG * nrows])
                    dst = out_d[gb * BG * 128:(gb + 1) * BG * 128,
                                r0:r0 + nrows].rearrange(
                                    "(i p) c -> p i c", p=128)
                    nc.sync.dma_start(
                        dst, ost[:, 0:BG * nrows].rearrange(
                            "p (i c) -> p i c", c=nrows))

                # ---- main scan ----
                done_chunks = 0
                for t in range(horizon):
                    g, s = divmod(t, 4)
                    g2, s2 = divmod(t + 1, 4)
                    ts_ = sgroups[g]
                    if g2 > g:
                        alloc_group(g2)
                    stash = {}

                    def stage1(j):
                        c0, c1 = j * NTILE, (j + 1) * NTILE
                        ph1 = psA.tile([128, 2 * NTILE], f32, tag="ph1",
                                       name="ph1")
                        for m in range(2):
                            nc.tensor.matmul(
                                ph1[:, m * NTILE:(m + 1) * NTILE],
                                w1sb[32 * s:32 * s + F,
                                     m * 128:(m + 1) * 128],
                                ts_[32 * s:32 * s + F, c0:c1],
                                start=True, stop=True,
                                tile_position=(32 * s, 0))
                        h1t = h1p.tile([128, 2 * NTILE], f32r, tag="h1")
                        nc.scalar.activation(h1t[:, :], ph1[:, :], Tanh)
                        stash[j] = h1t

                    def stage2(j):
                        h1t = stash.pop(j)
                        ph2 = psB.tile([128, 2 * NTILE], f32, tag="ph2",
                                       name="ph2")
                        for m in range(2):
                            nc.tensor.matmul(
                                ph2[:, m * NTILE:(m + 1) * NTILE],
                                w2k0[:, m * 128:(m + 1) * 128],
                                h1t[:, 0:NTILE], start=True, stop=False)
                            nc.tensor.matmul(
                                ph2[:, m * NTILE:(m + 1) * NTILE],
                                w2k1[:, m * 128:(m + 1) * 128],
                                h1t[:, NTILE:2 * NTILE], start=False, stop=True)
                        h2t = h2p.tile([128, 2 * NTILE], f32r, tag="h2")
                        nc.scalar.activation(h2t[:, :], ph2[:, :], Tanh)
                        stash[("h2", j)] = h2t

                    def stage3(j):
                        h2t = stash.pop(("h2", j))
                        if j % 2 == 0:
                            if pd_tag == "ph2":
                                pdt = psB.tile([128, 2 * NTILE], f32,
                                               tag="ph2", name="pd")
                            elif pd_tag == "ph1":
                                pdt = psA.tile([128, 2 * NTILE], f32,
                                               tag="ph1", name="pd")
                            else:
                                pdt = psS.tile([128, 2 * NTILE], f32,
                                               tag="sm", name="pd")
                            stash["pd"] = pdt
                        pdt = stash["pd"]
                        dcol = (j % 2) * NTILE
                        nc.tensor.matmul(pdt[0:NST, dcol:dcol + NTILE],
                                         w3sb[:, 0:NST], h2t[:, 0:NTILE],
                                         start=True, stop=False)
                        nc.tensor.matmul(pdt[0:NST, dcol:dcol + NTILE],
                                         w3sb[:, NST:2 * NST],
                                         h2t[:, NTILE:2 * NTILE],
                                         start=False, stop=True)
                        if j < 2:
                            p0, pw = j * NTILE, NTILE
                        elif j % 2 == 1 or j == nb - 1:
                            p0 = (j - j % 2) * NTILE
                            pw = (j % 2 + 1) * NTILE
                        else:
                            p0 = None
                        if p0 is not None:
                            # state(t+1) = state(t) + d (W3 pre-scaled by DT)
                            nc.vector.tensor_add(
                                sgroups[g2][32 * s2:32 * s2 + NST, p0:p0 + pw],
                                pdt[0:NST, (p0 % (2 * NTILE)):
                                    (p0 % (2 * NTILE)) + pw],
                                ts_[32 * s:32 * s + NST, p0:p0 + pw])

                    for j in range(nb):
                        stage1(j)
                        if j >= 1:
                            stage2(j - 1)
                            stage3(j - 1)
                    stage2(nb - 1)
                    stage3(nb - 1)

                    # record state(t+1) as output row t (DRAM staging)
                    nc.sync.dma_start(
                        hst_d[NST * t:NST * (t + 1), :],
                        sgroups[g2][32 * s2:32 * s2 + NST, :].bitcast(f32))

                    # interleave output transposes for completed chunks
                    if (done_chunks < len(chunks)
                            and t + 1 == cstart[done_chunks] + chunks[done_chunks]):
                        start_chunk(done_chunks)
                        done_chunks += 1
                    for _ in range(min(spread, len(pending))):
                        emit_block(*pending.pop(0))

                while done_chunks < len(chunks):
                    start_chunk(done_chunks)
                    done_chunks += 1
                while pending:
                    emit_block(*pending.pop(0))

    nc.compile()
    return nc


def _get_nc(b_core=B_CORE, horizon=H, **kw):
    key = (b_core, horizon, tuple(sorted(kw.items())))
    if key not in _CACHE:
        _CACHE[key] = _build(b_core, horizon, **kw)
    return _CACHE[key]


def _run(x, W1, b1, W2, b2, W3, b3, **spmd_kwargs):
    import concourse.bass_utils as bass_utils

    x = np.ascontiguousarray(np.asarray(x, dtype=np.float32))
    W1 = np.ascontiguousarray(np.asarray(W1, dtype=np.float32))
    W2 = np.ascontiguousarray(np.asarray(W2, dtype=np.float32))
    W3 = np.ascontiguousarray(np.asarray(W3, dtype=np.float32))
    for b in (b1, b2, b3):
        assert not np.any(np.asarray(b)), "kernel built for zero biases"

    nc = _get_nc()
    w3dt = np.ascontiguousarray(DT * W3)
    ident = np.eye(128, dtype=np.float32)
    xr = x.reshape(B_TOTAL, H * F)

    in_maps = []
    for c in range(N_CORES):
        in_maps.append({
            "x": xr[c * B_CORE:(c + 1) * B_CORE],
            "w1": W1, "w2": W2, "w3dt": w3dt, "ident": ident,
        })
    res = bass_utils.run_bass_kernel_spmd(nc, in_maps,
                                          core_ids=list(range(N_CORES)),
                                          **spmd_kwargs)
    out = np.concatenate(
        [res.results[c]["out"].reshape(B_CORE, H, NST) for c in range(N_CORES)],
        axis=0)
    return out, res


def kernel(x, W1, b1, W2, b2, W3, b3):
    out, _ = _run(x, W1, b1, W2, b2, W3, b3)
    return out


# revision 40
# speedup vs baseline: 104.0990x; 1.0017x over previous
"""Trainium2 Bass kernel for a 50-step autoregressive MLP rollout.

reference semantics (per batch row b):
    state = x[b, 0, 2:9]                       # 7 state vars
    for t in range(50):
        u = x[b, t, 0:2]                       # 2 controls
        h1 = tanh([u, state] @ W1 + b1)        # [9] -> [256]
        h2 = tanh(h1 @ W2 + b2)                # [256] -> [256]
        d  = h2 @ W3 + b3                      # [256] -> [7]
        state = state + 0.02 * d
        out[b, t] = state

Strategy: pure data parallel over batch across 8 NeuronCores (4096 rows each).
On-chip layout is feature-major ([feature, batch]) so the MLP weights are the
stationary matmul operand and the recurrence never transposes activations.
Matmuls run as float32r (full-rate fp32 path).  Each step's 9-feature input
vector lives in one quadrant slot of a [128, B] tile: state in rows 32s..32s+7
(written by the DVE update at an aligned base) and the two controls in rows
32s+7..32s+9 (written by DMA, which may target unaligned partitions; compute
engines may not).  The first MLP layer is then a single K=9 matmul per output
chunk.  Controls are transposed on-chip with the PE at startup and staged
through DRAM so each step's pair can be fetched to its slot with a plain DMA.
The state history is staged to DRAM per step and transposed back to
batch-major in chunks that overlap the scan.  b1/b2/b3 are zeros for this
problem (spec fill=zeros) and asserted so.
"""

import numpy as np

B_TOTAL = 32768
N_CORES = 8
B_CORE = B_TOTAL // N_CORES          # 4096
H = 50
F = 9
NCTRL = 2
NST = 7
HID = 256
DT = 0.02
NTILE = 512

_CACHE = {}


def _build(b_core=B_CORE, horizon=H, psa_bufs=1, psb_bufs=2, pd_tag="own", pt_tag="sm",
           chunks=(18, 18, 10, 4), spread=1, reps=1):
    import concourse.bacc as bacc
    import concourse.mybir as mybir
    import concourse.tile as tile

    f32 = mybir.dt.float32
    f32r = mybir.dt.float32r
    Tanh = mybir.ActivationFunctionType.Tanh

    nb = b_core // NTILE
    n_groups = (horizon + 4) // 4
    n_blk = b_core // 128
    xcols = horizon * F
    # output chunks (in steps); each must fit 128 sbuf rows (<=18)
    chunks = [c for c in chunks]
    while sum(chunks) > horizon:
        chunks[-1] -= 1
        if chunks[-1] == 0:
            chunks.pop()
    if sum(chunks) < horizon:
        chunks.append(horizon - sum(chunks))
    cstart = [sum(chunks[:i]) for i in range(len(chunks))]

    nc = bacc.Bacc("TRN2", target_bir_lowering=False, debug=False,
                   num_devices=N_CORES)

    x_d = nc.dram_tensor("x", [b_core, xcols], f32, kind="ExternalInput").ap()
    w1_d = nc.dram_tensor("w1", [F, HID], f32r, kind="ExternalInput").ap()
    w2_d = nc.dram_tensor("w2", [HID, HID], f32r, kind="ExternalInput").ap()
    w3_d = nc.dram_tensor("w3dt", [HID, NST], f32r, kind="ExternalInput").ap()
    id_d = nc.dram_tensor("ident", [128, 128], f32, kind="ExternalInput").ap()
    out_d = nc.dram_tensor("out", [b_core, horizon * NST], f32,
                           kind="ExternalOutput").ap()
    hst_d = nc.dram_tensor("hstage", [NST * horizon, b_core], f32,
                           kind="Internal").ap()
    ust_d = nc.dram_tensor("ustage", [128, b_core], f32r,
                           kind="Internal").ap()

    with tile.TileContext(nc) as tc:
        with (
            tc.tile_pool(name="persist", bufs=1) as pp,
            tc.tile_pool(name="xst", bufs=3) as xp,
            tc.tile_pool(name="sT", bufs=2) as sp_,
            tc.tile_pool(name="h1p", bufs=4) as h1p,
            tc.tile_pool(name="h2p", bufs=4) as h2p,
            tc.tile_pool(name="shb", bufs=2) as shp,
            tc.tile_pool(name="ostg", bufs=4) as op_,
            tc.tile_pool(name="psA", bufs=psa_bufs, space="PSUM") as psA,
            tc.tile_pool(name="psB", bufs=psb_bufs, space="PSUM") as psB,
            tc.tile_pool(name="psS", bufs=1, space="PSUM") as psS,
        ):
            w1sb = pp.tile([128, HID], f32r, tag="w1sb")
            w2k0 = pp.tile([128, HID], f32r, tag="w2k0")
            w2k1 = pp.tile([128, HID], f32r, tag="w2k1")
            w3sb = pp.tile([128, 2 * NST], f32r, tag="w3sb")
            ident = pp.tile([128, 128], f32, tag="ident")
            ut = pp.tile([128, b_core], f32r, tag="ut")  # controls.T row f*64+t

            for k in range(4):
                # per-quadrant W1, permuted to [state rows; control rows]
                nc.sync.dma_start(w1sb[32 * k:32 * k + NST, :], w1_d[NCTRL:F, :])
                nc.sync.dma_start(w1sb[32 * k + NST:32 * k + F, :],
                                  w1_d[0:NCTRL, :])
            nc.sync.dma_start(w2k0[:, :], w2_d[0:128, :])
            nc.sync.dma_start(w2k1[:, :], w2_d[128:256, :])
            nc.sync.dma_start(w3sb[:, 0:NST], w3_d[0:128, :])
            nc.sync.dma_start(w3sb[:, NST:2 * NST], w3_d[128:256, :])
            nc.sync.dma_start(ident[:, :], id_d[:, :])

            ust_v = ust_d.rearrange("(f t) b -> t f b", t=64)
            sgroups = [None] * (n_groups + 1)

            def fill_u(g):
                ts_ = sgroups[g]
                for s in range(4):
                    t = 4 * g + s
                    if t < horizon:
                        nc.sync.dma_start(
                            ts_[32 * s + NST:32 * s + F, :], ust_v[t])

            def alloc_group(g, fill=True):
                ts_ = sp_.tile([128, b_core], f32r, name=f"sT_g{g}", tag="sT")
                sgroups[g] = ts_
                if fill:
                    fill_u(g)
                return ts_

            for _rep in range(reps):
                st0 = alloc_group(0, fill=False)

                # ---- prologue: transpose controls (all t) and state0 ----
                bpd = min(4, n_blk)
                for q in range(n_blk // bpd):
                    rows = bpd * 128
                    cw = bpd * 128
                    xs = xp.tile([128, bpd * xcols], f32, tag="xs")
                    src = x_d[q * rows:(q + 1) * rows, :].rearrange(
                        "(j p) c -> p j c", p=128)
                    nc.sync.dma_start(
                        xs[:, :].rearrange("p (j c) -> p j c", c=xcols), src)
                    pu = psB.tile([128, 2 * NTILE], f32, tag="ph2", name="pu")
                    for fi in range(NCTRL):
                        for j in range(bpd):
                            xv = xs[:, j * xcols:(j + 1) * xcols].rearrange(
                                "p (t f) -> p t f", f=F)
                            nc.tensor.transpose(
                                pu[0:horizon,
                                   fi * NTILE + j * 128:fi * NTILE + (j + 1) * 128],
                                xv[:, :, fi], ident[:, :])
                    ps0 = psS.tile([128, NTILE], f32, tag="sm", name="ps0")
                    for j in range(bpd):
                        nc.tensor.transpose(
                            ps0[0:NST, j * 128:(j + 1) * 128],
                            xs[:, j * xcols + NCTRL:j * xcols + F],
                            ident[:, :])
                    for fi in range(NCTRL):
                        nc.vector.tensor_copy(
                            ut[64 * fi:64 * fi + horizon, q * cw:(q + 1) * cw],
                            pu[0:horizon, fi * NTILE:fi * NTILE + cw])
                    nc.vector.tensor_copy(st0[0:NST, q * cw:(q + 1) * cw],
                                          ps0[0:NST, 0:cw])
                for fi in range(NCTRL):
                    nc.sync.dma_start(ust_d[64 * fi:64 * fi + horizon, :],
                                      ut[64 * fi:64 * fi + horizon, :])
                fill_u(0)

                # ---- epilogue task queue: (chunk, blk) transposes ----
                pending = []
                shbs = {}

                BG = 4  # blocks per transpose group (BG*nrows <= 512: one PSUM bank)

                def start_chunk(k):
                    r0, nrows = cstart[k] * NST, chunks[k] * NST
                    shb = shp.tile([128, b_core], f32, tag="shb",
                                   name=f"shb{k}")
                    nc.sync.dma_start(shb[0:nrows, :],
                                      hst_d[r0:r0 + nrows, :])
                    shbs[k] = shb
                    pending.extend((k, gb) for gb in range(n_blk // BG))

                def emit_block(k, gb):
                    r0, nrows = cstart[k] * NST, chunks[k] * NST
                    shb = shbs[k]
                    ptp = {"sm": psS, "ph1": psA, "ph2": psB}[pt_tag]
                    pt = ptp.tile([128, 2 * NTILE], f32, tag=pt_tag, name="pt")
                    for i in range(BG):
                        blk = gb * BG + i
                        nc.tensor.transpose(
                            pt[0:128, i * nrows:(i + 1) * nrows],
                            shb[0:nrows, blk * 128:(blk + 1) * 128],
                            ident[0:nrows, 0:nrows])
                    ost = op_.tile([128, BG * 128], f32, tag="ost")
                    nc.vector.tensor_copy(ost[:, 0:BG * nrows],
                                          pt[0:128, 0:BG * nrows])
                    dst = out_d[gb * BG * 128:(gb + 1) * BG * 128,
                                r0:r0 + nrows].rearrange(
                                    "(i p) c -> p i c", p=128)
                    nc.sync.dma_start(
                        dst, ost[:, 0:BG * nrows].rearrange(
                            "p (i c) -> p i c", c=nrows))

                # ---- main scan ----
                done_chunks = 0
                for t in range(horizon):
                    g, s = divmod(t, 4)
                    g2, s2 = divmod(t + 1, 4)
                    ts_ = sgroups[g]
                    if g2 > g:
                        alloc_group(g2)
                    stash = {}

                    def stage1(j):
                        c0, c1 = j * NTILE, (j + 1) * NTILE
                        ph1 = psA.tile([128, 2 * NTILE], f32, tag="ph1",
                                       name="ph1")
                        for m in range(2):
                            nc.tensor.matmul(
                                ph1[:, m * NTILE:(m + 1) * NTILE],
                                w1sb[32 * s:32 * s + F,
                                     m * 128:(m + 1) * 128],
                                ts_[32 * s:32 * s + F, c0:c1],
                                start=True, stop=True,
                                tile_position=(32 * s, 0))
                        h1t = h1p.tile([128, 2 * NTILE], f32r, tag="h1")
                        nc.scalar.activation(h1t[:, :], ph1[:, :], Tanh)
                        stash[j] = h1t

                    def stage2(j):
                        h1t = stash.pop(j)
                        ph2 = psB.tile([128, 2 * NTILE], f32, tag="ph2",
                                       name="ph2")
                        for m in range(2):
                            nc.tensor.matmul(
                                ph2[:, m * NTILE:(m + 1) * NTILE],
                                w2k0[:, m * 128:(m + 1) * 128],
                                h1t[:, 0:NTILE], start=True, stop=False)
                            nc.tensor.matmul(
                                ph2[:, m * NTILE:(m + 1) * NTILE],
                                w2k1[:, m * 128:(m + 1) * 128],
                                h1t[:, NTILE:2 * NTILE], start=False, stop=True)
                        h2t = h2p.tile([128, 2 * NTILE], f32r, tag="h2")
                        nc.scalar.activation(h2t[:, :], ph2[:, :], Tanh)
                        stash[("h2", j)] = h2t

                    def stage3(j):
                        h2t = stash.pop(("h2", j))
                        if j % 2 == 0:
                            if pd_tag == "ph2":
                                pdt = psB.tile([128, 2 * NTILE], f32,
                                               tag="ph2", name="pd")
                            elif pd_tag == "ph1":
                                pdt = psA.tile([128, 2 * NTILE], f32,
                                               tag="ph1", name="pd")
                            else:
                                pdt = psS.tile([128, 2 * NTILE], f32,
                                               tag="sm", name="pd")
                            stash["pd"] = pdt
                        pdt = stash["pd"]
                        dcol = (j % 2) * NTILE
                        nc.tensor.matmul(pdt[0:NST, dcol:dcol + NTILE],
                                         w3sb[:, 0:NST], h2t[:, 0:NTILE],
                                         start=True, stop=False)
                        nc.tensor.matmul(pdt[0:NST, dcol:dcol + NTILE],
                                         w3sb[:, NST:2 * NST],
                                         h2t[:, NTILE:2 * NTILE],
                                         start=False, stop=True)
                        if j < 2:
                            p0, pw = j * NTILE, NTILE
                        elif j % 2 == 1 or j == nb - 1:
                            p0 = (j - j % 2) * NTILE
                            pw = (j % 2 + 1) * NTILE
                        else:
                            p0 = None
                        if p0 is not None:
                            # state(t+1) = state(t) + d (W3 pre-scaled by DT)
                            nc.vector.tensor_add(
                                sgroups[g2][32 * s2:32 * s2 + NST, p0:p0 + pw],
                                pdt[0:NST, (p0 % (2 * NTILE)):
                                    (p0 % (2 * NTILE)) + pw],
                                ts_[32 * s:32 * s + NST, p0:p0 + pw])

                    for j in range(nb):
                        stage1(j)
                        if j >= 1:
                            stage2(j - 1)
                            stage3(j - 1)
                    stage2(nb - 1)
                    stage3(nb - 1)

                    # record state(t+1) as output row t (DRAM staging)
                    nc.sync.dma_start(
                        hst_d[NST * t:NST * (t + 1), :],
                        sgroups[g2][32 * s2:32 * s2 + NST, :].bitcast(f32))

                    # interleave output transposes for completed chunks
                    if (done_chunks < len(chunks)
                            and t + 1 == cstart[done_chunks] + chunks[done_chunks]):
                        start_chunk(done_chunks)
                        done_chunks += 1
                    for _ in range(min(spread, len(pending))):
                        emit_block(*pending.pop(0))

                while done_chunks < len(chunks):
                    start_chunk(done_chunks)
                    done_chunks += 1
                while pending:
                    emit_block(*pending.pop(0))

    nc.compile()
    return nc


def _get_nc(b_core=B_CORE, horizon=H, **kw):
    key = (b_core, horizon, tuple(sorted(kw.items())))
    if key not in _CACHE:
        _CACHE[key] = _build(b_core, horizon, **kw)
    return _CACHE[key]


def _run(x, W1, b1, W2, b2, W3, b3, **spmd_kwargs):
    import concourse.bass_utils as bass_utils

    x = np.ascontiguousarray(np.asarray(x, dtype=np.float32))
    W1 = np.ascontiguousarray(np.asarray(W1, dtype=np.float32))
    W2 = np.ascontiguousarray(np.asarray(W2, dtype=np.float32))
    W3 = np.ascontiguousarray(np.asarray(W3, dtype=np.float32))
    for b in (b1, b2, b3):
        assert not np.any(np.asarray(b)), "kernel built for zero biases"

    nc = _get_nc()
    w3dt = np.ascontiguousarray(DT * W3)
    ident = np.eye(128, dtype=np.float32)
    xr = x.reshape(B_TOTAL, H * F)

    in_maps = []
    for c in range(N_CORES):
        in_maps.append({
            "x": xr[c * B_CORE:(c + 1) * B_CORE],
            "w1": W1, "w2": W2, "w3dt": w3dt, "ident": ident,
        })
    res = bass_utils.run_bass_kernel_spmd(nc, in_maps,
                                          core_ids=list(range(N_CORES)),
                                          **spmd_kwargs)
    out = np.concatenate(
        [res.results[c]["out"].reshape(B_CORE, H, NST) for c in range(N_CORES)],
        axis=0)
    return out, res


def kernel(x, W1, b1, W2, b2, W3, b3):
    out, _ = _run(x, W1, b1, W2, b2, W3, b3)
    return out
